# revision 7
# baseline (speedup 1.0000x reference)
"""MoE feed-forward (top-2 of 8 experts, SwiGLU) on 8 Trainium2 NeuronCores.

Strategy: expert-parallel. Core c holds expert c's weights (bf16) and the full
token set (x replicated). Each core:
  1. computes gate logits for all tokens in fp32 on the PE,
  2. derives its own expert's renormalized top-2 combine weight per token,
  3. runs the expert FFN densely over all tokens in bf16 (fp32 accumulate),
  4. scales by the combine weight (zero for non-selected tokens),
  5. ReduceScatters the [D, N] output across the 8 cores.
Host only reshapes/packs inputs and concatenates/transposes the output.

Shapes (hardcoded per the problem spec):
  x [2, 2048, 1024], gate_w [8, 1024], w1/w3 [8, 2816, 1024], w2 [8, 1024, 2816]
"""

import sys

sys.path.insert(0, "/opt/trn_rl_repo")

import numpy as np
import ml_dtypes

B, S, D, H, E = 2, 2048, 1024, 2816, 8
N = B * S                    # 4096 tokens
NCORES = 8
NCH = 8                      # token chunks
TCH = N // NCH               # 512 tokens per chunk
DK = D // 128                # 8 contraction tiles over D
HI = H // 128                # 22 tiles over H
DI = D // 128                # 8 output tiles over D

_CACHE = {}


def _build_program(with_collective=True, reps=1):
    import concourse.mybir as mybir
    from concourse import bacc, tile
    from concourse.bass import ts
    from concourse.masks import make_identity

    F32 = mybir.dt.float32
    BF16 = mybir.dt.bfloat16
    Alu = mybir.AluOpType
    Act = mybir.ActivationFunctionType

    nc = bacc.Bacc("TRN2", target_bir_lowering=False, debug=False,
                   num_devices=NCORES)

    xg_d = nc.dram_tensor("xg", [NCH, 128, DK, TCH], F32, kind="ExternalInput")
    xb_d = nc.dram_tensor("xb", [NCH, 128, DK, TCH], BF16, kind="ExternalInput")
    gw_d = nc.dram_tensor("gw", [128, DK, E], F32, kind="ExternalInput")
    es_d = nc.dram_tensor("esel", [128, E], F32, kind="ExternalInput")
    w1_d = nc.dram_tensor("w1p", [HI, 128, DK, 128], BF16, kind="ExternalInput")
    w3_d = nc.dram_tensor("w3p", [HI, 128, DK, 128], BF16, kind="ExternalInput")
    w2_d = nc.dram_tensor("w2p", [128, HI, DI, 128], BF16, kind="ExternalInput")
    out_d = nc.dram_tensor("out", [128, N], F32, kind="ExternalOutput")

    with tile.TileContext(nc) as tc:
        with (
            tc.tile_pool(name="const", bufs=1) as cp,
            tc.tile_pool(name="xg", bufs=2) as xgp,
            tc.tile_pool(name="xb", bufs=2) as xbp,
            tc.tile_pool(name="wst", bufs=3) as wst,
            tc.tile_pool(name="gt", bufs=2) as gtp,
            tc.tile_pool(name="sm", bufs=3) as sm,
            tc.tile_pool(name="yt", bufs=3) as ytp,
            tc.tile_pool(name="pg", bufs=2, space="PSUM") as pg,
            tc.tile_pool(name="ph", bufs=2, space="PSUM") as ph,
            tc.tile_pool(name="py", bufs=2, space="PSUM") as py,
            tc.tile_pool(name="dram", bufs=2, space="DRAM") as dr,
        ):
            # ---- constants ----
            w2_sb = cp.tile([128, HI, DI, 128], BF16)
            nc.sync.dma_start(w2_sb[:], w2_d[:])
            gw_sb = cp.tile([128, DK, E], F32)
            nc.sync.dma_start(gw_sb[:], gw_d[:])
            esel_sb = cp.tile([128, E], F32)
            nc.sync.dma_start(esel_sb[:], es_d[:])
            ident = cp.tile([128, 128], F32)
            make_identity(nc, ident[:])

            for ch in [c for _ in range(reps) for c in range(NCH)]:
                # ---- load x chunk (fp32 for gate, bf16 for FFN) ----
                xg_t = xgp.tile([128, DK, TCH], F32, tag="xg")
                nc.sync.dma_start(xg_t[:], xg_d[ch])
                xb_t = xbp.tile([128, DK, TCH], BF16, tag="xb")
                nc.sync.dma_start(xb_t[:], xb_d[ch])

                # ---- gate: logitsT [E, TCH] in fp32 ----
                lg_ps = pg.tile([E, TCH], F32, tag="g")
                for dk in range(DK):
                    nc.tensor.matmul(lg_ps[:], gw_sb[:, dk, :], xg_t[:, dk, :],
                                     start=(dk == 0), stop=(dk == DK - 1))
                lg_sb = sm.tile([E, TCH], F32, tag="lg")
                nc.vector.tensor_copy(lg_sb[:], lg_ps[:])

                # per-128-token tile: transpose to [128, E], top-2 softmax
                wcol = sm.tile([128, TCH // 128], F32, tag="wcol")
                for tt in range(TCH // 128):
                    tp_ps = pg.tile([128, E], F32, tag="g")
                    nc.tensor.transpose(tp_ps[:], lg_sb[:, ts(tt, 128)],
                                        ident[:E, :E])
                    lt = sm.tile([128, E], F32, tag="lt")
                    nc.vector.tensor_copy(lt[:], tp_ps[:])
                    mx = sm.tile([128, 8], F32, tag="mx")
                    nc.vector.max(mx[:], lt[:])
                    m1n = sm.tile([128, 1], F32, tag="m1n")
                    nc.vector.tensor_scalar_mul(m1n[:], mx[:, 0:1], -1.0)
                    # e2 = exp(m2 - m1)
                    e2 = sm.tile([128, 1], F32, tag="e2")
                    nc.scalar.activation(e2[:], mx[:, 1:2], Act.Exp,
                                         bias=m1n[:, 0:1])
                    # l_c = <logits, esel>
                    lcs = sm.tile([128, E], F32, tag="lcs")
                    lc = sm.tile([128, 1], F32, tag="lc")
                    nc.vector.tensor_tensor(lcs[:], lt[:], esel_sb[:], Alu.mult)
                    nc.vector.tensor_reduce(lc[:], lcs[:],
                                            mybir.AxisListType.X, Alu.add)
                    # selected iff l_c >= second max
                    sel = sm.tile([128, 1], F32, tag="sel")
                    nc.vector.tensor_tensor(sel[:], lc[:], mx[:, 1:2], Alu.is_ge)
                    ec = sm.tile([128, 1], F32, tag="ec")
                    nc.scalar.activation(ec[:], lc[:], Act.Exp, bias=m1n[:, 0:1])
                    den = sm.tile([128, 1], F32, tag="den")
                    nc.vector.tensor_scalar_add(den[:], e2[:], 1.0)
                    rden = sm.tile([128, 1], F32, tag="rden")
                    nc.vector.reciprocal(rden[:], den[:])
                    num = sm.tile([128, 1], F32, tag="num")
                    nc.vector.tensor_tensor(num[:], ec[:], sel[:], Alu.mult)
                    nc.vector.tensor_tensor(wcol[:, tt:tt + 1], num[:], rden[:],
                                            Alu.mult)

                # ---- broadcast combine weights to [128, TCH] ----
                W_sb = sm.tile([128, TCH], F32, tag="W")
                for tt in range(TCH // 128):
                    wt_ps = pg.tile([1, 128], F32, tag="g")
                    nc.tensor.transpose(wt_ps[:], wcol[:, tt:tt + 1], ident[:])
                    wrow = sm.tile([1, 128], F32, tag="wrow")
                    nc.vector.tensor_copy(wrow[:], wt_ps[:])
                    nc.gpsimd.partition_broadcast(W_sb[:, ts(tt, 128)],
                                                  wrow[0:1, :])

                # ---- FFN: gT[h, t] = silu(w1 xT) * (w3 xT), bf16 ----
                gt_t = gtp.tile([128, HI, TCH], BF16, tag="gt")
                for hi in range(HI):
                    w1_t = wst.tile([128, DK, 128], BF16, tag="w1")
                    nc.sync.dma_start(w1_t[:], w1_d[hi])
                    w3_t = wst.tile([128, DK, 128], BF16, tag="w3")
                    nc.sync.dma_start(w3_t[:], w3_d[hi])
                    h1_ps = ph.tile([128, TCH], F32, tag="h1")
                    h3_ps = ph.tile([128, TCH], F32, tag="h3")
                    for dk in range(DK):
                        nc.tensor.matmul(h1_ps[:], w1_t[:, dk, :],
                                         xb_t[:, dk, :],
                                         start=(dk == 0), stop=(dk == DK - 1))
                    for dk in range(DK):
                        nc.tensor.matmul(h3_ps[:], w3_t[:, dk, :],
                                         xb_t[:, dk, :],
                                         start=(dk == 0), stop=(dk == DK - 1))
                    sig = sm.tile([128, TCH], F32, tag="sig")
                    nc.scalar.activation(sig[:], h1_ps[:], Act.Sigmoid)
                    sil = sm.tile([128, TCH], F32, tag="sil")
                    nc.vector.tensor_tensor(sil[:], sig[:], h1_ps[:], Alu.mult)
                    nc.vector.tensor_tensor(gt_t[:, hi, :], sil[:], h3_ps[:],
                                            Alu.mult)

                # ---- yT[d, t] = w2 gT, scaled by combine weights ----
                ytc = dr.tile([DI, 128, TCH], F32, tag="ytc")
                for di in range(DI):
                    y_ps = py.tile([128, TCH], F32, tag="y")
                    for hi in range(HI):
                        nc.tensor.matmul(y_ps[:], w2_sb[:, hi, di, :],
                                         gt_t[:, hi, :],
                                         start=(hi == 0), stop=(hi == HI - 1))
                    yt_t = ytp.tile([128, TCH], F32, tag="yt")
                    nc.vector.tensor_tensor(yt_t[:], y_ps[:], W_sb[:], Alu.mult)
                    nc.sync.dma_start(ytc[di], yt_t[:])

                # ---- combine across experts: ReduceScatter over 8 cores ----
                if with_collective:
                    rso = dr.tile([128, TCH], F32, tag="rso")
                    nc.gpsimd.collective_compute(
                        "ReduceScatter",
                        mybir.AluOpType.add,
                        replica_groups=[list(range(NCORES))],
                        ins=[ytc[:].opt()],
                        outs=[rso[:].opt()],
                    )
                    nc.sync.dma_start(out_d[:, ts(ch, TCH)], rso[:])
                else:
                    nc.sync.dma_start(out_d[:, ts(ch, TCH)], ytc[0])

    nc.compile()
    return nc


def _get_program():
    if "nc" not in _CACHE:
        _CACHE["nc"] = _build_program()
    return _CACHE["nc"]


def _pack_inputs(x, gate_w, w1, w2, w3):
    """Host-side layout packing (no math beyond dtype casts)."""
    bf16 = ml_dtypes.bfloat16
    xt = np.ascontiguousarray(np.asarray(x, dtype=np.float32).reshape(N, D).T)
    # [dk, d, ch, t] -> [ch, d, dk, t]
    xg = np.ascontiguousarray(
        xt.reshape(DK, 128, NCH, TCH).transpose(2, 1, 0, 3))
    xb = xg.astype(bf16)
    gw = np.ascontiguousarray(
        np.asarray(gate_w, dtype=np.float32).T.reshape(DK, 128, E)
        .transpose(1, 0, 2))
    w1 = np.asarray(w1, dtype=np.float32)
    w2 = np.asarray(w2, dtype=np.float32)
    w3 = np.asarray(w3, dtype=np.float32)

    in_maps = []
    for c in range(NCORES):
        esel = np.zeros((128, E), dtype=np.float32)
        esel[:, c] = 1.0
        w1p = np.ascontiguousarray(
            w1[c].reshape(HI, 128, DK, 128).transpose(0, 3, 2, 1)).astype(bf16)
        w3p = np.ascontiguousarray(
            w3[c].reshape(HI, 128, DK, 128).transpose(0, 3, 2, 1)).astype(bf16)
        w2p = np.ascontiguousarray(
            w2[c].reshape(DI, 128, HI, 128).transpose(3, 2, 0, 1)).astype(bf16)
        in_maps.append({
            "xg": xg, "xb": xb, "gw": gw, "esel": esel,
            "w1p": w1p, "w3p": w3p, "w2p": w2p,
        })
    return in_maps


def _unpack_output(results):
    yT = np.concatenate([results[c]["out"] for c in range(NCORES)], axis=0)
    return np.ascontiguousarray(yT.T).reshape(B, S, D).astype(np.float32)


def kernel(x, gate_w, w1, w2, w3):
    from concourse import bass_utils

    nc = _get_program()
    in_maps = _pack_inputs(x, gate_w, w1, w2, w3)
    res = bass_utils.run_bass_kernel_spmd(nc, in_maps,
                                          core_ids=list(range(NCORES)))
    return _unpack_output(res.results)
